# revision 8
# baseline (speedup 1.0000x reference)
"""Inverse DTCWT (biort bandpass) level-1 reconstruction as a Bass/Tile kernel.

Math: the reference is
    y = M0c @ Yl @ M0r' + M1c @ LH @ M0r' + M0c @ HL @ M1r' + M2c @ HH @ M2r'
where M* are 256x256 banded matrices (1D taps + symmetric padding folded in)
and LH/HL/HH are the c2q quad-interleaves of subband pairs (0,5)/(2,3)/(1,4).

All c2q sums/differences and every layout shuffle run on the HOST (numpy);
the device sees three bf16 streams:
  tb:  per pair, [top rows; bot rows] halves stacked across partitions so a
       single 128-contraction matmul applies both the even-row and odd-row
       column-filter taps in one pass,
  yl:  the lowpass image split in two 128-row chunks,
  flt: all banded filter matrices pre-sliced to their nonzero column extents.
Because the filters are banded (13/19 taps -> halfwidth 6/9), each matmul's
moving extent is ~134-138 columns instead of 256 - nearly halving PE streaming
time vs dense 256-wide passes. Region-split accumulation into one PSUM bank is
legal because start=True clears has_written for the whole bank and start=False
matmuls overwrite-where-clear / accumulate-where-set per element.

Everything is bf16 (inputs, weights, z intermediates, output; PSUM stays
fp32): halves DMA traffic vs fp32 and enables Fast Weight Load (2x faster
LDWEIGHTS than fp32). rel-err vs fp32 reference ~3e-3.

Sharding: pure data parallel, batch dim (8) across 8 cores.
"""
import sys

if "/opt/trn_rl_repo" not in sys.path:
    sys.path.insert(0, "/opt/trn_rl_repo")

import numpy as np
import ml_dtypes

BF16 = ml_dtypes.bfloat16

_C, _H = 64, 256  # channels per core, image size
_NCORES = 8
_G = 4            # images (channels) per group
_NG = _C // _G    # 16 groups

# pair q -> (band1, band2, col-filter id); filter ids: 0=g0o(13) 1=g1o(19) 2=g2o(13)
_PAIRS = [(0, 5, 1), (1, 4, 2), (2, 3, 0)]
# stage B: (z index, row-filter id) in emission order
_ROWMAP = [(0, 0), (2, 1), (1, 2)]
_HALF = {0: 6, 1: 9, 2: 6}  # filter halfwidths (L//2)


def _ext(m):
    """Even-aligned (lo, hi) output-column extents for a halfwidth-m band
    matrix split at row 128: lo rows 0:128 touch cols [0, 128+m),
    hi rows 128:256 touch cols [128-m, 256)."""
    lo_end = 128 + m
    lo_end += lo_end % 2
    hi_start = 128 - m
    hi_start -= hi_start % 2
    return (0, lo_end), (hi_start, 256)


def _flt_layout():
    """Static layout of the packed filter tensor [128, T]:
    entries keyed (kind, idx, half) -> (offset, h0, width)."""
    lay, off = {}, 0
    def add(key, m):
        nonlocal off
        (l0, l1), (h0, h1) = _ext(m)
        for half, (a, b) in ((0, (l0, l1)), (1, (h0, h1))):
            lay[key + (half,)] = (off, a, b - a)
            off += b - a
    for q, (_, _, f) in enumerate(_PAIRS):
        add(("A", q), _HALF[f])
    add(("YL", 0), _HALF[0])
    for p, (_, f) in enumerate(_ROWMAP):
        add(("B", p), _HALF[f])
    return lay, off


_FLT_LAY, _FLT_T = _flt_layout()


def _band_matrix(h, N):
    """M @ x == colfilter(x, h) with symmetric padding, in float64."""
    h = np.asarray(h, dtype=np.float64)
    L = h.shape[0]
    m = L // 2
    A = np.zeros((N, N), dtype=np.float64)
    for i in range(N):
        for k in range(L):
            s = i + k - m
            if s < 0:
                s = -1 - s
            elif s >= N:
                s = 2 * N - 1 - s
            A[i, s] += h[L - 1 - k]
    return A


def build_consts(g0o, g1o, g2o):
    """Pack every filter block into one [128, T] bf16 tensor."""
    Ms = [_band_matrix(g, _H) for g in (g0o, g1o, g2o)]
    s2 = np.sqrt(2.0)
    flt = np.zeros((128, _FLT_T), dtype=np.float64)

    def put(key, block):
        off, h0, w = _FLT_LAY[key]
        assert block.shape == (128, w), (key, block.shape, w)
        flt[:, off:off + w] = block

    for q, (_, _, f) in enumerate(_PAIRS):
        MT = Ms[f].T  # [src_row, out_col]
        ReT, RoT = MT[0::2] / s2, MT[1::2] / s2  # [128, 256]
        for half in (0, 1):
            off, h0, w = _FLT_LAY[("A", q, half)]
            sl = slice(64 * half, 64 * half + 64)
            put(("A", q, half), np.vstack([ReT[sl], RoT[sl]])[:, h0:h0 + w])
    M0T = Ms[0].T
    for half in (0, 1):
        off, h0, w = _FLT_LAY[("YL", 0, half)]
        put(("YL", 0, half), M0T[128 * half:128 * half + 128, h0:h0 + w])
    for p, (_, f) in enumerate(_ROWMAP):
        MT = Ms[f].T
        for half in (0, 1):
            off, h0, w = _FLT_LAY[("B", p, half)]
            put(("B", p, half), MT[128 * half:128 * half + 128, h0:h0 + w])
    return {"flt": flt.astype(BF16)}


def build_nc(n_images):
    import concourse.bacc as bacc
    import concourse.mybir as mybir
    from concourse.tile import TileContext

    f32 = mybir.dt.float32
    bf16 = mybir.dt.bfloat16
    nc = bacc.Bacc(None, target_bir_lowering=False, debug=False)

    ng = n_images // _G
    assert ng * _G == n_images
    tb_d = nc.declare_dram_parameter(
        "tbp", [ng, 128, _G, 3, 2, 256], bf16, isOutput=False
    )
    yl_d = nc.declare_dram_parameter(
        "ylp", [ng, 128, _G, 2, 256], bf16, isOutput=False
    )
    flt_d = nc.declare_dram_parameter("flt", [128, _FLT_T], bf16, isOutput=False)
    out_d = nc.declare_dram_parameter(
        "out", [ng, 128, _G, 2, 256], bf16, isOutput=True
    )

    def fslice(flt_sb, key):
        off, h0, w = _FLT_LAY[key]
        return flt_sb[:, off:off + w], h0, w

    with TileContext(nc) as tc:
        with (
            tc.tile_pool(name="consts", bufs=1) as cpool,
            tc.tile_pool(name="io", bufs=2) as io_pool,
            tc.tile_pool(name="zsb", bufs=2) as z_pool,
            tc.tile_pool(name="ps", bufs=2, space="PSUM") as ps_pool,
        ):
            flt = cpool.tile([128, _FLT_T], bf16)
            nc.sync.dma_start(flt[:], flt_d[:])

            def stage_a(tb, yl, i):
                """Emit stage A of one image; returns its bf16 zsb tile."""
                z = [
                    ps_pool.tile([128, 2, 256], f32, tag=f"z{q}", name=f"z{q}")
                    for q in range(3)
                ]
                for q in range(3):
                    for cc in range(2):
                        cs = slice(128 * cc, 128 * cc + 128)
                        for half in (0, 1):
                            mv, h0, w = fslice(flt, ("A", q, half))
                            nc.tensor.matmul(
                                z[q][:, cc, h0:h0 + w],
                                tb[:, i, q, half, cs],
                                mv,
                                start=(cc == 0 and half == 0),
                                stop=(q != 0 and cc == 1 and half == 1),
                            )
                    if q == 0:
                        # lowpass path accumulates into z[0]
                        for cc in range(2):
                            cs = slice(128 * cc, 128 * cc + 128)
                            for k in range(2):
                                mv, h0, w = fslice(flt, ("YL", 0, k))
                                nc.tensor.matmul(
                                    z[0][:, cc, h0:h0 + w],
                                    yl[:, i, k, cs],
                                    mv,
                                    start=False,
                                    stop=(cc == 1 and k == 1),
                                )
                zsb = z_pool.tile([128, 3, 2, 256], bf16, tag="zsb")
                nc.vector.tensor_copy(out=zsb[:, 0], in_=z[0][:])
                nc.scalar.copy(zsb[:, 2], z[2][:])
                nc.vector.tensor_copy(out=zsb[:, 1], in_=z[1][:])
                return zsb

            def stage_b(zsb, out_sb, i):
                """Emit stage B of one image into out_sb[:, i]."""
                yp = ps_pool.tile([128, 2, 256], f32, tag="yp", name="yp")
                for r in range(2):
                    rs = slice(128 * r, 128 * r + 128)
                    for p, (zi, _) in enumerate(_ROWMAP):
                        for cc in range(2):
                            mv, h0, w = fslice(flt, ("B", p, cc))
                            nc.tensor.matmul(
                                yp[:, r, h0:h0 + w],
                                zsb[:, zi, cc, rs],
                                mv,
                                start=(r == 0 and p == 0 and cc == 0),
                                stop=(r == 1 and p == 2 and cc == 1),
                            )
                nc.scalar.copy(out_sb[:, i, :, :], yp[:])

            # Software-pipelined: stage B of image t-1 is emitted after
            # stage A of image t, so the PSUM->SBUF copies of t-1 hide
            # under A(t)'s matmuls and the PE never idles between stages.
            pend = None  # (zsb, out_sb, i, g) awaiting stage B
            tiles = {}
            for t in range(n_images):
                g, i = divmod(t, _G)
                if i == 0:
                    tb = io_pool.tile(
                        [128, _G, 3, 2, 256], bf16, tag="tb", bufs=8, name="tb"
                    )
                    yl = io_pool.tile(
                        [128, _G, 2, 256], bf16, tag="yl", bufs=8, name="yl"
                    )
                    # per-image DMA slices: image (g, 0) becomes runnable
                    # after ~1/4 of the group's bytes have landed
                    for j in range(_G):
                        nc.sync.dma_start(tb[:, j], tb_d[g][:, j])
                        nc.sync.dma_start(yl[:, j], yl_d[g][:, j])
                    out_sb = io_pool.tile(
                        [128, _G, 2, 256], bf16, tag="out_sb", name="out_sb"
                    )
                    tiles[g] = (tb, yl, out_sb)
                zsb = stage_a(tiles[g][0], tiles[g][1], i)
                if pend is not None:
                    pg, pi = pend[3], pend[2]
                    stage_b(pend[0], pend[1], pi)
                    nc.scalar.dma_start(out_d[pg][:, pi], tiles[pg][2][:, pi])
                    if pi == _G - 1:
                        del tiles[pg]
                pend = (zsb, tiles[g][2], i, g)
            stage_b(pend[0], pend[1], pend[2])
            nc.scalar.dma_start(
                out_d[pend[3]][:, pend[2]], tiles[pend[3]][2][:, pend[2]]
            )
    nc.compile()
    return nc


_NC_CACHE = {}


def _get_nc(n_images):
    if n_images not in _NC_CACHE:
        _NC_CACHE[n_images] = build_nc(n_images)
    return _NC_CACHE[n_images]


def pack_inputs(Yl_k, Yhr_k, Yhi_k):
    """Per-core repack (c2q on host) into bf16 group-major layouts.

    tbp[g, p, i, q, s, c]: pair-q c2q data for channel 4g+i; partitions hold
      [top rows 64s:64s+64 ; bot rows 64s:64s+64] stacked; c = 2w + (r/i).
    ylp[g, p, i, k, w] = Yl[4g+i, 128k+p, w]
    """
    C = Yl_k.shape[0]
    ng = C // _G
    tbp = np.empty((ng, 128, _G, 3, 2, 256), dtype=BF16)
    for q, (b1, b2, _) in enumerate(_PAIRS):
        w1r, w1i = Yhr_k[:, b1], Yhi_k[:, b1]   # [C, 128, 128]
        w2r, w2i = Yhr_k[:, b2], Yhi_k[:, b2]
        top = np.empty((C, 128, 256), dtype=np.float32)
        bot = np.empty((C, 128, 256), dtype=np.float32)
        top[:, :, 0::2] = w1r + w2r
        top[:, :, 1::2] = w1i + w2i
        bot[:, :, 0::2] = w1i - w2i
        bot[:, :, 1::2] = w2r - w1r
        for s in range(2):
            hs = slice(64 * s, 64 * s + 64)
            # [C, 128, 256] -> [ng, G, 128, 256] -> [ng, 128, G, 256]
            stk = np.concatenate([top[:, hs], bot[:, hs]], axis=1)
            tbp[:, :, :, q, s, :] = (
                stk.reshape(ng, _G, 128, 256).transpose(0, 2, 1, 3)
            )
    ylp = np.ascontiguousarray(
        Yl_k.reshape(ng, _G, 2, 128, 256).transpose(0, 3, 1, 2, 4)
    ).astype(BF16)
    return tbp, ylp


def unpack_output(outp):
    """outp (ng, 128, G, 2, 256): [g, p, i, k, w] = y[G*g+i, 128k+p, w]."""
    return np.ascontiguousarray(
        outp.astype(np.float32).transpose(0, 2, 3, 1, 4).reshape(-1, 256, 256)
    )


def kernel(Yl, Yhr, Yhi, g0o, g1o, g2o):
    from concourse.bass_utils import run_bass_kernel_spmd

    Yl = np.asarray(Yl, dtype=np.float32)
    Yhr = np.asarray(Yhr, dtype=np.float32)
    Yhi = np.asarray(Yhi, dtype=np.float32)
    consts = build_consts(np.asarray(g0o), np.asarray(g1o), np.asarray(g2o))

    nc = _get_nc(_C)
    in_maps = []
    for k in range(_NCORES):
        tbp, ylp = pack_inputs(Yl[k], Yhr[k], Yhi[k])
        in_maps.append({"ylp": ylp, "tbp": tbp, **consts})
    res = run_bass_kernel_spmd(nc, in_maps, list(range(_NCORES)))
    out = np.stack([unpack_output(res.results[k]["out"]) for k in range(_NCORES)])
    return out.astype(np.float32)


# revision 10
# speedup vs baseline: 1.1329x; 1.1329x over previous
"""Inverse DTCWT (biort bandpass) level-1 reconstruction as a Bass/Tile kernel.

Math: the reference is
    y = M0c @ Yl @ M0r' + M1c @ LH @ M0r' + M0c @ HL @ M1r' + M2c @ HH @ M2r'
where M* are 256x256 banded matrices (1D taps + symmetric padding folded in)
and LH/HL/HH are the c2q quad-interleaves of subband pairs (0,5)/(2,3)/(1,4).

All c2q sums/differences and every layout shuffle run on the HOST (numpy);
the device sees three bf16 streams:
  tb:  per pair, [top rows; bot rows] halves stacked across partitions so a
       single 128-contraction matmul applies both the even-row and odd-row
       column-filter taps in one pass,
  yl:  the lowpass image split in two 128-row chunks,
  flt: all banded filter matrices pre-sliced to their nonzero column extents.
Because the filters are banded (13/19 taps -> halfwidth 6/9), each matmul's
moving extent is ~134-138 columns instead of 256 - nearly halving PE streaming
time vs dense 256-wide passes. Region-split accumulation into one PSUM bank is
legal because start=True clears has_written for the whole bank and start=False
matmuls overwrite-where-clear / accumulate-where-set per element.

Everything is bf16 (inputs, weights, z intermediates, output; PSUM stays
fp32): halves DMA traffic vs fp32 and enables Fast Weight Load (2x faster
LDWEIGHTS than fp32). rel-err vs fp32 reference ~3e-3.

Sharding: pure data parallel, batch dim (8) across 8 cores.
"""
import sys

if "/opt/trn_rl_repo" not in sys.path:
    sys.path.insert(0, "/opt/trn_rl_repo")

import numpy as np
import ml_dtypes

BF16 = ml_dtypes.bfloat16

_C, _H = 64, 256  # channels per core, image size
_NCORES = 8
_G = 4            # images (channels) per group
_NG = _C // _G    # 16 groups

# pair q -> (band1, band2, col-filter id); filter ids: 0=g0o(13) 1=g1o(19) 2=g2o(13)
_PAIRS = [(0, 5, 1), (1, 4, 2), (2, 3, 0)]
# stage B: (z index, row-filter id) in emission order
_ROWMAP = [(0, 0), (2, 1), (1, 2)]
_HALF = {0: 6, 1: 9, 2: 6}  # filter halfwidths (L//2)


def _ext(m):
    """Even-aligned (lo, hi) output-column extents for a halfwidth-m band
    matrix split at row 128: lo rows 0:128 touch cols [0, 128+m),
    hi rows 128:256 touch cols [128-m, 256)."""
    lo_end = 128 + m
    lo_end += lo_end % 2
    hi_start = 128 - m
    hi_start -= hi_start % 2
    return (0, lo_end), (hi_start, 256)


def _flt_layout():
    """Static layout of the packed filter tensor [128, T]:
    entries keyed (kind, idx, half) -> (offset, h0, width)."""
    lay, off = {}, 0
    def add(key, m):
        nonlocal off
        (l0, l1), (h0, h1) = _ext(m)
        for half, (a, b) in ((0, (l0, l1)), (1, (h0, h1))):
            lay[key + (half,)] = (off, a, b - a)
            off += b - a
    for q, (_, _, f) in enumerate(_PAIRS):
        add(("A", q), _HALF[f])
    add(("YL", 0), _HALF[0])
    for p, (_, f) in enumerate(_ROWMAP):
        add(("B", p), _HALF[f])
    return lay, off


_FLT_LAY, _FLT_T = _flt_layout()


def _band_matrix(h, N):
    """M @ x == colfilter(x, h) with symmetric padding, in float64."""
    h = np.asarray(h, dtype=np.float64)
    L = h.shape[0]
    m = L // 2
    A = np.zeros((N, N), dtype=np.float64)
    for i in range(N):
        for k in range(L):
            s = i + k - m
            if s < 0:
                s = -1 - s
            elif s >= N:
                s = 2 * N - 1 - s
            A[i, s] += h[L - 1 - k]
    return A


def build_consts(g0o, g1o, g2o):
    """Pack every filter block into one [128, T] bf16 tensor."""
    Ms = [_band_matrix(g, _H) for g in (g0o, g1o, g2o)]
    s2 = np.sqrt(2.0)
    flt = np.zeros((128, _FLT_T), dtype=np.float64)

    def put(key, block):
        off, h0, w = _FLT_LAY[key]
        assert block.shape == (128, w), (key, block.shape, w)
        flt[:, off:off + w] = block

    for q, (_, _, f) in enumerate(_PAIRS):
        MT = Ms[f].T  # [src_row, out_col]
        ReT, RoT = MT[0::2] / s2, MT[1::2] / s2  # [128, 256]
        for half in (0, 1):
            off, h0, w = _FLT_LAY[("A", q, half)]
            sl = slice(64 * half, 64 * half + 64)
            put(("A", q, half), np.vstack([ReT[sl], RoT[sl]])[:, h0:h0 + w])
    M0T = Ms[0].T
    for half in (0, 1):
        off, h0, w = _FLT_LAY[("YL", 0, half)]
        put(("YL", 0, half), M0T[128 * half:128 * half + 128, h0:h0 + w])
    for p, (_, f) in enumerate(_ROWMAP):
        MT = Ms[f].T
        for half in (0, 1):
            off, h0, w = _FLT_LAY[("B", p, half)]
            put(("B", p, half), MT[128 * half:128 * half + 128, h0:h0 + w])
    return {"flt": flt.astype(BF16)}


def build_nc(n_images):
    import concourse.bacc as bacc
    import concourse.mybir as mybir
    from concourse.tile import TileContext

    f32 = mybir.dt.float32
    bf16 = mybir.dt.bfloat16
    nc = bacc.Bacc(None, target_bir_lowering=False, debug=False)

    ng = n_images // _G
    assert ng * _G == n_images
    tb_d = nc.declare_dram_parameter(
        "tbp", [ng, 128, _G, 3, 2, 256], bf16, isOutput=False
    )
    yl_d = nc.declare_dram_parameter(
        "ylp", [ng, 128, _G, 2, 256], bf16, isOutput=False
    )
    flt_d = nc.declare_dram_parameter("flt", [128, _FLT_T], bf16, isOutput=False)
    out_d = nc.declare_dram_parameter(
        "out", [ng, 128, _G, 2, 256], bf16, isOutput=True
    )

    def fslice(flt_sb, key):
        off, h0, w = _FLT_LAY[key]
        return flt_sb[:, off:off + w], h0, w

    with TileContext(nc) as tc:
        with (
            tc.tile_pool(name="consts", bufs=1) as cpool,
            tc.tile_pool(name="io", bufs=2) as io_pool,
            tc.tile_pool(name="zsb", bufs=2) as z_pool,
            tc.tile_pool(name="ps", bufs=2, space="PSUM") as ps_pool,
        ):
            flt = cpool.tile([128, _FLT_T], bf16)
            nc.sync.dma_start(flt[:], flt_d[:])

            def stage_a(tb, yl, i):
                """Emit stage A of one image; returns its bf16 zsb tile."""
                z = [
                    ps_pool.tile([128, 2, 256], f32, tag=f"z{q}", name=f"z{q}")
                    for q in range(3)
                ]
                for q in range(3):
                    for cc in range(2):
                        cs = slice(128 * cc, 128 * cc + 128)
                        for half in (0, 1):
                            mv, h0, w = fslice(flt, ("A", q, half))
                            nc.tensor.matmul(
                                z[q][:, cc, h0:h0 + w],
                                tb[:, i, q, half, cs],
                                mv,
                                start=(cc == 0 and half == 0),
                                stop=(q != 0 and cc == 1 and half == 1),
                            )
                    if q == 0:
                        # lowpass path accumulates into z[0]
                        for cc in range(2):
                            cs = slice(128 * cc, 128 * cc + 128)
                            for k in range(2):
                                mv, h0, w = fslice(flt, ("YL", 0, k))
                                nc.tensor.matmul(
                                    z[0][:, cc, h0:h0 + w],
                                    yl[:, i, k, cs],
                                    mv,
                                    start=False,
                                    stop=(cc == 1 and k == 1),
                                )
                zsb = z_pool.tile([128, 3, 2, 256], bf16, tag="zsb")
                nc.vector.tensor_copy(out=zsb[:, 0], in_=z[0][:])
                nc.scalar.copy(zsb[:, 2], z[2][:])
                nc.vector.tensor_copy(out=zsb[:, 1], in_=z[1][:])
                return zsb

            def stage_b(zsb, out_sb, i):
                """Emit stage B of one image into out_sb[:, i]."""
                yp = ps_pool.tile([128, 2, 256], f32, tag="yp", name="yp")
                for r in range(2):
                    rs = slice(128 * r, 128 * r + 128)
                    for p, (zi, _) in enumerate(_ROWMAP):
                        for cc in range(2):
                            mv, h0, w = fslice(flt, ("B", p, cc))
                            nc.tensor.matmul(
                                yp[:, r, h0:h0 + w],
                                zsb[:, zi, cc, rs],
                                mv,
                                start=(r == 0 and p == 0 and cc == 0),
                                stop=(r == 1 and p == 2 and cc == 1),
                            )
                nc.scalar.copy(out_sb[:, i, :, :], yp[:])

            # Software-pipelined: stage B of image t-1 is emitted after
            # stage A of image t, so the PSUM->SBUF copies of t-1 hide
            # under A(t)'s matmuls and the PE never idles between stages.
            pend = None  # (zsb, out_sb, i, g) awaiting stage B
            tiles = {}
            for t in range(n_images):
                g, i = divmod(t, _G)
                if i == 0:
                    tb = io_pool.tile(
                        [128, _G, 3, 2, 256], bf16, tag="tb", bufs=8, name="tb"
                    )
                    yl = io_pool.tile(
                        [128, _G, 2, 256], bf16, tag="yl", bufs=8, name="yl"
                    )
                    if g == 0:
                        # group 0 lands per image so the first matmul can
                        # start after ~1/4 of the group's bytes
                        for j in range(_G):
                            nc.sync.dma_start(tb[:, j], tb_d[g][:, j])
                            nc.sync.dma_start(yl[:, j], yl_d[g][:, j])
                    else:
                        nc.sync.dma_start(tb[:], tb_d[g])
                        nc.sync.dma_start(yl[:], yl_d[g])
                    out_sb = io_pool.tile(
                        [128, _G, 2, 256], bf16, tag="out_sb", name="out_sb"
                    )
                    tiles[g] = (tb, yl, out_sb)
                zsb = stage_a(tiles[g][0], tiles[g][1], i)
                if pend is not None:
                    pg, pi = pend[3], pend[2]
                    stage_b(pend[0], pend[1], pi)
                    if pi == _G - 1:
                        nc.scalar.dma_start(out_d[pg], tiles[pg][2][:])
                        del tiles[pg]
                pend = (zsb, tiles[g][2], i, g)
            stage_b(pend[0], pend[1], pend[2])
            nc.scalar.dma_start(out_d[pend[3]], tiles[pend[3]][2][:])
    nc.compile()
    return nc


_NC_CACHE = {}


def _get_nc(n_images):
    if n_images not in _NC_CACHE:
        _NC_CACHE[n_images] = build_nc(n_images)
    return _NC_CACHE[n_images]


def pack_inputs(Yl_k, Yhr_k, Yhi_k):
    """Per-core repack (c2q on host) into bf16 group-major layouts.

    tbp[g, p, i, q, s, c]: pair-q c2q data for channel 4g+i; partitions hold
      [top rows 64s:64s+64 ; bot rows 64s:64s+64] stacked; c = 2w + (r/i).
    ylp[g, p, i, k, w] = Yl[4g+i, 128k+p, w]
    """
    C = Yl_k.shape[0]
    ng = C // _G
    tbp = np.empty((ng, 128, _G, 3, 2, 256), dtype=BF16)
    for q, (b1, b2, _) in enumerate(_PAIRS):
        w1r, w1i = Yhr_k[:, b1], Yhi_k[:, b1]   # [C, 128, 128]
        w2r, w2i = Yhr_k[:, b2], Yhi_k[:, b2]
        top = np.empty((C, 128, 256), dtype=np.float32)
        bot = np.empty((C, 128, 256), dtype=np.float32)
        top[:, :, 0::2] = w1r + w2r
        top[:, :, 1::2] = w1i + w2i
        bot[:, :, 0::2] = w1i - w2i
        bot[:, :, 1::2] = w2r - w1r
        for s in range(2):
            hs = slice(64 * s, 64 * s + 64)
            # [C, 128, 256] -> [ng, G, 128, 256] -> [ng, 128, G, 256]
            stk = np.concatenate([top[:, hs], bot[:, hs]], axis=1)
            tbp[:, :, :, q, s, :] = (
                stk.reshape(ng, _G, 128, 256).transpose(0, 2, 1, 3)
            )
    ylp = np.ascontiguousarray(
        Yl_k.reshape(ng, _G, 2, 128, 256).transpose(0, 3, 1, 2, 4)
    ).astype(BF16)
    return tbp, ylp


def unpack_output(outp):
    """outp (ng, 128, G, 2, 256): [g, p, i, k, w] = y[G*g+i, 128k+p, w]."""
    return np.ascontiguousarray(
        outp.astype(np.float32).transpose(0, 2, 3, 1, 4).reshape(-1, 256, 256)
    )


def kernel(Yl, Yhr, Yhi, g0o, g1o, g2o):
    from concourse.bass_utils import run_bass_kernel_spmd

    Yl = np.asarray(Yl, dtype=np.float32)
    Yhr = np.asarray(Yhr, dtype=np.float32)
    Yhi = np.asarray(Yhi, dtype=np.float32)
    consts = build_consts(np.asarray(g0o), np.asarray(g1o), np.asarray(g2o))

    nc = _get_nc(_C)
    in_maps = []
    for k in range(_NCORES):
        tbp, ylp = pack_inputs(Yl[k], Yhr[k], Yhi[k])
        in_maps.append({"ylp": ylp, "tbp": tbp, **consts})
    res = run_bass_kernel_spmd(nc, in_maps, list(range(_NCORES)))
    out = np.stack([unpack_output(res.results[k]["out"]) for k in range(_NCORES)])
    return out.astype(np.float32)
